# revision 52
# baseline (speedup 1.0000x reference)
"""Block-causal attention (B=4, N=2048, C=1024, H=16, block=128) on 8 TRN2 NeuronCores.

Sharding: core = 2*b + g  (b in 0..3 batches, g in 0..1 head-groups of 8 heads).
Each core:
  - computes q^T,k^T (feature-major) and v (token-major) for its batch/head-group
    from a host-pre-transposed x^T and head-sliced w_qkv  (no duplicated FLOPs),
  - block-causal attention: q-tile i attends to k-tiles 0..i (no masking needed,
    128-token blocks align with tiles),
  - partial out-projection with its 512-row slice of w_proj.
Host sums the two partial projections per batch and adds b_proj.

All PE operands are bf16 (inputs host-cast); accumulation is fp32 in PSUM.
K=64 / M=64 matmul pairs overlap on the PE's half-array tiles, so the S^T
head-pair and the AV/rowsum col-pairs each cost ~one N-column stream.

The attention j-loop is software-pipelined 2 deep and batched in j-pairs so
same-shape matmuls stay contiguous (each PE tile-config switch costs ~100ns):
  PE:     ..., S(j),S(j+1), AVSM(j-2),AVSM(j-1), [fillers], S(j+2), ...
  Scalar: ..., exp(j), exp(j+1), ...
Fillers are single matmuls of the next chunk's QKV chains / previous chunks'
projection chains, drained with a deterministic even spread so no PE work is
left for a serial tail.
"""

import numpy as np
import ml_dtypes
from collections import deque
from contextlib import ExitStack

B, N, C, H, HD = 4, 2048, 1024, 16, 64
HPC = 8               # heads per core
F = HPC * HD          # 512 features per core
NCORES = 8
SCALE = float(HD) ** -0.5
NT = N // 128         # 16 token tiles
NCH = 4               # token chunks of 512

_CACHE = {}


def _build():
    import concourse.mybir as mybir
    import concourse.tile as tile
    from concourse import bacc

    f32 = mybir.dt.float32
    bf16 = mybir.dt.bfloat16
    Exp = mybir.ActivationFunctionType.Exp

    nc = bacc.Bacc("TRN2", target_bir_lowering=False, debug=False,
                   num_devices=NCORES)

    xT = nc.dram_tensor("xT", [C, N], bf16, kind="ExternalInput")
    wq = nc.dram_tensor("wq", [C, F], bf16, kind="ExternalInput")
    wk = nc.dram_tensor("wk", [C, F], bf16, kind="ExternalInput")
    wv = nc.dram_tensor("wv", [C, F], bf16, kind="ExternalInput")
    wp = nc.dram_tensor("wp", [F, C], bf16, kind="ExternalInput")
    ones_d = nc.dram_tensor("ones", [128, 64], bf16, kind="ExternalInput")
    out = nc.dram_tensor("out", [N, C], bf16, kind="ExternalOutput")

    with tile.TileContext(nc) as tc, ExitStack() as ctx:
        persist = ctx.enter_context(tc.tile_pool(name="persist", bufs=1))
        xt_pool = ctx.enter_context(tc.tile_pool(name="xt", bufs=3))
        qt_pool = ctx.enter_context(tc.tile_pool(name="qt", bufs=3))
        at_pool = ctx.enter_context(tc.tile_pool(name="attnT", bufs=6))
        exp_pool = ctx.enter_context(tc.tile_pool(name="expT", bufs=8))
        rc_pool = ctx.enter_context(tc.tile_pool(name="recip", bufs=3))
        ost_pool = ctx.enter_context(tc.tile_pool(name="ost", bufs=4))
        ps_mm = ctx.enter_context(tc.tile_pool(name="ps_mm", bufs=2, space="PSUM"))
        ps_s = ctx.enter_context(tc.tile_pool(name="ps_s", bufs=2, space="PSUM"))
        ps_av = ctx.enter_context(tc.tile_pool(name="ps_av", bufs=1, space="PSUM"))
        ps_sum = ctx.enter_context(tc.tile_pool(name="ps_sum", bufs=1, space="PSUM"))

        # ---- persistent weights ----
        wq_t = [persist.tile([128, F], bf16, name=f"wq{kk}", tag=f"wq{kk}") for kk in range(8)]
        wk_t = [persist.tile([128, F], bf16, name=f"wk{kk}", tag=f"wk{kk}") for kk in range(8)]
        wv_t = [persist.tile([128, F], bf16, name=f"wv{kk}", tag=f"wv{kk}") for kk in range(8)]
        wp_t = [persist.tile([128, C], bf16, name=f"wp{kk}", tag=f"wp{kk}") for kk in range(4)]
        ones_t = persist.tile([128, 64], bf16, name="ones", tag="ones")

        # persistent k^T (per head-pair per chunk) and v (per token tile)
        kt_t = [[persist.tile([128, 512], bf16, name=f"kT{hp}_{jc}", tag=f"kT{hp}_{jc}")
                 for jc in range(NCH)] for hp in range(4)]
        v_t = [persist.tile([128, F], bf16, name=f"v{t}", tag=f"v{t}") for t in range(NT)]

        def load_weights():
            # spread startup-critical loads across the 3 DMA queues by the
            # time each tile is first needed (xt/wq ~now, wk next, wv later);
            # chunk-0's xt is itself split sync/gpsimd in xt_load_step
            nc.gpsimd.dma_start(ones_t[:], ones_d[:])
            for kk in range(5):
                nc.scalar.dma_start(wq_t[kk][:], wq[kk * 128:(kk + 1) * 128, :])
            for kk in range(5, 8):
                nc.sync.dma_start(wq_t[kk][:], wq[kk * 128:(kk + 1) * 128, :])
            for kk in range(8):
                nc.gpsimd.dma_start(wk_t[kk][:], wk[kk * 128:(kk + 1) * 128, :])
            for kk in range(4):
                nc.gpsimd.dma_start(wv_t[kk][:], wv[kk * 128:(kk + 1) * 128, :])
            for kk in range(4, 8):
                nc.scalar.dma_start(wv_t[kk][:], wv[kk * 128:(kk + 1) * 128, :])
            for kk in range(4):
                nc.gpsimd.dma_start(wp_t[kk][:], wp[kk * 128:(kk + 1) * 128, :])

        qt_state = {c: {} for c in range(NCH)}   # qt_state[c][hp] = tile
        at_state = {c: {} for c in range(NCH)}   # at_state[c][hp] = tile
        xt_state = {}                            # xt_state[c] = [8 tiles]
        kt_done = {c: set() for c in range(NCH)}  # hp whose k-chain is emitted
        v_done = {c: 0 for c in range(NCH)}       # emitted v-chains per chunk

        # ---------- fine-grained QKV / proj filler steps ----------
        def xt_load_step(c):
            def emit():
                c0 = c * 512
                tiles = []
                for kk in range(8):
                    xt = xt_pool.tile([128, 512], bf16, name=f"xt{kk}",
                                      tag=f"xt{kk}")
                    q = nc.gpsimd if (c == 0 and kk >= 4) else nc.sync
                    q.dma_start(xt[:],
                                xT[kk * 128:(kk + 1) * 128, c0:c0 + 512])
                    tiles.append(xt)
                xt_state[c] = tiles
            return emit

        def qk_chain_steps(c, hp, which):
            """8 single-matmul steps (+ finishing copy) for q^T/k^T of (c, hp)."""
            w_t = wq_t if which == 'q' else wk_t
            st = {}
            steps = []
            for kk in range(8):
                def step(kk=kk):
                    if kk == 0:
                        st['ps'] = ps_mm.tile([128, 512], f32, name="mm", tag="mm")
                    nc.tensor.matmul(st['ps'][:],
                                     w_t[kk][:, hp * 128:(hp + 1) * 128],
                                     xt_state[c][kk][:],
                                     start=(kk == 0), stop=(kk == 7))
                    if kk == 7:
                        if which == 'q':
                            qt = qt_pool.tile([128, 512], bf16, name=f"qT{hp}",
                                              tag=f"qT{hp}")
                            nc.vector.tensor_copy(qt[:], st['ps'][:])
                            qt_state[c][hp] = qt
                        else:
                            nc.vector.tensor_copy(kt_t[hp][c][:], st['ps'][:])
                            kt_done[c].add(hp)
                steps.append(step)
            return steps

        def v_chain_steps(c, tl):
            t = 4 * c + tl
            st = {}
            steps = []
            for kk in range(8):
                def step(kk=kk):
                    if kk == 0:
                        st['ps'] = ps_mm.tile([128, 512], f32, name="mm", tag="mm")
                    nc.tensor.matmul(st['ps'][:],
                                     xt_state[c][kk][:, tl * 128:(tl + 1) * 128],
                                     wv_t[kk][:],
                                     start=(kk == 0), stop=(kk == 7))
                    if kk == 7:
                        nc.vector.tensor_copy(v_t[t][:], st['ps'][:])
                        v_done[c] += 1
                steps.append(step)
            return steps

        def qkv_steps(c):
            steps = [xt_load_step(c)]
            steps += qk_chain_steps(c, 0, 'q') + qk_chain_steps(c, 0, 'k')
            for tl in range(4):
                steps += v_chain_steps(c, tl)
            for hp in range(1, 4):
                steps += qk_chain_steps(c, hp, 'q') + qk_chain_steps(c, hp, 'k')
            return steps

        def proj_chain_steps(c, tl, n2):
            t = 4 * c + tl
            st = {}
            steps = []
            for kk in range(4):
                def step(kk=kk):
                    if kk == 0:
                        st['ps'] = ps_mm.tile([128, 512], f32, name="mm", tag="mm")
                    nc.tensor.matmul(
                        st['ps'][:],
                        at_state[c][kk][:, tl * 128:(tl + 1) * 128],
                        wp_t[kk][:, n2 * 512:(n2 + 1) * 512],
                        start=(kk == 0), stop=(kk == 3))
                    if kk == 3:
                        ost = ost_pool.tile([128, 512], bf16, name="ost", tag="ost")
                        # final chunk's copies run in the tail where both
                        # ScalarE and DVE are idle: alternate so consecutive
                        # chains' PSUM-freeing copies run in parallel; earlier
                        # chunks overlap exp, keep those on DVE
                        if c == NCH - 1 and (2 * tl + n2) % 2 == 0:
                            nc.scalar.copy(ost[:], st['ps'][:])
                        else:
                            nc.vector.tensor_copy(ost[:], st['ps'][:])
                        nc.sync.dma_start(
                            out[t * 128:(t + 1) * 128, n2 * 512:(n2 + 1) * 512],
                            ost[:])
                steps.append(step)
            return steps

        def proj_steps(c):
            steps = []
            for tl in range(4):
                for n2 in range(2):
                    steps += proj_chain_steps(c, tl, n2)
            return steps

        # ---------- attention primitives ----------
        def s_pair(c, hp, j, ss):
            jd = j - 4 * c
            vco = jd * 128 if jd > 0 else 0
            kt = kt_t[hp][j // 4]
            kc = (j % 4) * 128
            qt_c = qt_state[c][hp]
            nc.tensor.matmul(ss[:, vco:512],
                             kt[0:64, kc:kc + 128],
                             qt_c[0:64, vco:512],
                             start=True, stop=True)
            nc.tensor.matmul(ss[:, 512 + vco:1024],
                             kt[64:128, kc:kc + 128],
                             qt_c[64:128, vco:512],
                             start=True, stop=True)

        def exp_step(c, j, ss):
            jd = j - 4 * c
            vco = jd * 128 if jd > 0 else 0
            et = exp_pool.tile([128, 1024], bf16, name="e", tag="e")
            if vco:
                in3 = ss[:].rearrange("p (b q) -> p b q", b=2)[:, :, vco:512]
                out3 = et[:].rearrange("p (b q) -> p b q", b=2)[:, :, vco:512]
                nc.scalar.activation(out3, in3, Exp, scale=SCALE)
            else:
                nc.scalar.activation(et[:], ss[:], Exp, scale=SCALE)
            return et

        def avsm_step(c, hp, j, njt, et, av, sm):
            jd = j - 4 * c
            vco = jd * 128 if jd > 0 else 0
            first, last = (j == 0), (j == njt - 1)
            nc.tensor.matmul(av[0:64, vco:512],
                             v_t[j][:, hp * 128:hp * 128 + 64],
                             et[:, vco:512],
                             start=first, stop=last)
            nc.tensor.matmul(av[64:128, vco:512],
                             v_t[j][:, hp * 128 + 64:hp * 128 + 128],
                             et[:, 512 + vco:1024],
                             start=first, stop=last)
            nc.tensor.matmul(sm[0:64, vco:512],
                             ones_t[:, 0:64],
                             et[:, vco:512],
                             start=first, stop=last)
            nc.tensor.matmul(sm[64:128, vco:512],
                             ones_t[:, 0:64],
                             et[:, 512 + vco:1024],
                             start=first, stop=last)

        def finish_unit(c, hp, av, sm):
            rc = rc_pool.tile([128, 512], f32, name="recip", tag="recip")
            nc.vector.reciprocal_approx_fast(rc[:], sm[:])
            at = at_pool.tile([128, 512], bf16, name=f"at{hp}", tag=f"at{hp}")
            nc.vector.tensor_mul(at[:], av[:], rc[:])
            at_state[c][hp] = at

        # ---------- upfront: chunk-0 minimal QKV prefix ----------
        steps0 = qkv_steps(0)
        steps0[0]()                      # xt loads first
        load_weights()
        # q0 + k0 chains (what attention unit (0,0)'s S side needs),
        # interleaved per kk so the DMA-paced startup has no head-of-line
        # stalls (wq and wk arrive on different queues in parallel)
        for q_s, k_s in zip(steps0[1:9], steps0[9:17]):
            q_s()
            k_s()
        fillers = deque(steps0[1 + 16:])  # v chains + q/k for hp 1..3

        # ---------- software-pipelined attention over all (c, hp, j) ----------
        DEPTH = 4        # S/exp run this many j-steps ahead of AVSM

        pend = deque()   # (c, hp, j, njt, et, av, sm, is_last_of_unit)
        # total j-steps in the whole schedule, for even-spread filler pacing
        rem_j = sum(4 * c + 4 for c in range(NCH)) * 4

        def emit_fillers(n):
            for _ in range(n):
                if not fillers:
                    return
                fillers.popleft()()

        def pop_pending():
            p = pend.popleft()
            # the AVSM reads v_t of chunk p[0]: those chains must be emitted
            # first (the S/exp side only needs q/k, so it can start earlier)
            while v_done[p[0]] < 4:
                fillers.popleft()()
            avsm_step(*p[:7])
            if p[7]:
                finish_unit(p[0], p[1], p[5], p[6])
                if p[0] < NCH - 1 and p[1] == 3:
                    # chunk p[0] fully attended: its projection can fill
                    fillers.extend(proj_steps(p[0]))

        def unit_ready(c, hp):
            return hp in qt_state[c] and hp in kt_done[c]

        for c in range(NCH):
            if c < NCH - 1:
                fillers.extend(qkv_steps(c + 1))
            for hp in range(4):
                while not unit_ready(c, hp):
                    fillers.popleft()()
                njt = 4 * c + 4
                av = ps_av.tile([128, 512], f32, name="av", tag="av")
                sm = ps_sum.tile([128, 512], f32, name="sum", tag="sum")
                for j0 in range(0, njt, 2):
                    # j-pair batch: keeps same-shape matmuls contiguous so the
                    # PE pays ~1.5 tile-config switches per j instead of 3-4
                    for j in (j0, j0 + 1):
                        ss = ps_s.tile([128, 1024], f32, name="s", tag="s")
                        s_pair(c, hp, j, ss)
                        et_j = exp_step(c, j, ss)
                        pend.append((c, hp, j, njt, et_j, av, sm,
                                     j == njt - 1))
                    while len(pend) > DEPTH:
                        pop_pending()
                    # deterministic even-spread drain: no filler tail at the end
                    nf = -(-2 * len(fillers) // rem_j) if fillers else 0
                    emit_fillers(nf)
                    rem_j -= 2
        # drain pending AVSMs
        while pend:
            pop_pending()

        # drain remaining fillers, then the final chunk's projection
        while fillers:
            fillers.popleft()()
        for s in proj_steps(NCH - 1):
            s()

    nc.compile()
    return nc


def _get_nc():
    if "nc" not in _CACHE:
        _CACHE["nc"] = _build()
    return _CACHE["nc"]


def _in_maps(x, w_qkv, w_proj):
    wr = w_qkv.reshape(C, 3, H, HD)
    wpr = w_proj.reshape(H, HD, C)
    maps = []
    for core in range(NCORES):
        b, g = core // 2, core % 2
        hs = slice(g * HPC, (g + 1) * HPC)
        maps.append({
            "xT": np.ascontiguousarray(x[b].T).astype(ml_dtypes.bfloat16),
            "wq": np.ascontiguousarray(wr[:, 0, hs, :].reshape(C, F)).astype(ml_dtypes.bfloat16),
            "wk": np.ascontiguousarray(wr[:, 1, hs, :].reshape(C, F)).astype(ml_dtypes.bfloat16),
            "wv": np.ascontiguousarray(wr[:, 2, hs, :].reshape(C, F)).astype(ml_dtypes.bfloat16),
            "wp": np.ascontiguousarray(wpr[hs].reshape(F, C)).astype(ml_dtypes.bfloat16),
            "ones": np.ones((128, 64), dtype=ml_dtypes.bfloat16),
        })
    return maps


def kernel(x, w_qkv, w_proj, b_proj, _trace=False):
    from concourse.bass_utils import run_bass_kernel_spmd

    x = np.asarray(x, dtype=np.float32)
    w_qkv = np.asarray(w_qkv, dtype=np.float32)
    w_proj = np.asarray(w_proj, dtype=np.float32)
    b_proj = np.asarray(b_proj, dtype=np.float32)

    nc = _get_nc()
    in_maps = _in_maps(x, w_qkv, w_proj)
    try:
        res = run_bass_kernel_spmd(nc, in_maps, list(range(NCORES)),
                                   trace=_trace)
    except Exception:
        # Device may be wedged from a prior run; reset the axon-side NRT
        # and retry once.
        try:
            import ctypes
            import jax
            lib = ctypes.CDLL("/opt/axon/libaxon_pjrt.so")
            jax.devices()
            lib.axon_reset.restype = ctypes.c_int64
            lib.axon_reset()
        except Exception:
            pass
        res = run_bass_kernel_spmd(nc, in_maps, list(range(NCORES)),
                                   trace=_trace)
    out = np.empty((B, N, C), dtype=np.float32)
    for b in range(B):
        out[b] = (res.results[2 * b]["out"].astype(np.float32)
                  + res.results[2 * b + 1]["out"].astype(np.float32))
    out += b_proj.reshape(1, 1, C)
    if _trace:
        return out, res
    return out
